# revision 1
# baseline (speedup 1.0000x reference)
"""Trainium2 kernel for nn_DisjointSet: pointer-jump densify + value gather.

kernel(father, values) -> (f_conv, gathered), matching the reference:
    f_conv  = fixed point of f <- f[f]   (root id per point)
    gathered = values[f_conv]

Implementation: 8-core SPMD Bass kernel. Each core owns a contiguous
2M-point slice. Pointer doubling: J1 gathers from the replicated father
table; each subsequent jump all-gathers the updated slices (ncfw
AllGather) and gathers from the refreshed full table. Random access uses
indirect DMA row-gathers (128 offsets/instruction, one per SBUF
partition — the only per-element-random primitive walrus lowers
correctly). A final pass gathers values[f_conv].

The jump count for exact convergence is verified host-side after the
run (fixed-point + path-consistency checks using only input/output); on
failure the kernel is rebuilt with more jumps and rerun.
"""

import time

import numpy as np

import concourse.bass as bass
import concourse.mybir as mybir

P = 128
N_CORES = 8
N_EXPECTED = 16_777_216
DEFAULT_JUMPS = 4  # exact fixed point for the 2^24 reference forest


def _enable_dynamic_dma():
    """walrus in this container defaults to DynamicDMA disabled; the
    vector_dynamic_offsets DGE level is required for indirect DMA
    (per-partition dynamic offsets). Inject the flag into the walrus
    command line."""
    import concourse.bass_utils as bu

    if getattr(bu, "_dyn_dma_patched", False):
        return
    orig = bu.run_command

    def patched(cmd, *a, **kw):
        if cmd and isinstance(cmd[0], str) and "walrus_driver" in cmd[0]:
            if not any(str(c).startswith("--dge-levels") for c in cmd):
                cmd = list(cmd) + [
                    "--dge-levels=io,spill_reload,scalar_dynamic_offset,vector_dynamic_offsets",
                ]
        return orig(cmd, *a, **kw)

    bu.run_command = patched
    bu._dyn_dma_patched = True


def _build_kernel(N, B, W, n_jumps):
    """One SPMD program. N: table size; B: per-core slice; W: gather
    instructions per tile iteration; n_jumps: jump passes (>=1)."""
    assert B % (P * W) == 0
    NT = B // (P * W)
    nc = bass.Bass()

    father = nc.declare_dram_parameter("father", [N], mybir.dt.int32, isOutput=False)
    values = nc.declare_dram_parameter("values", [N], mybir.dt.float32, isOutput=False)
    fslice = nc.declare_dram_parameter("fslice", [B], mybir.dt.int32, isOutput=False)
    froot_out = nc.declare_dram_parameter("froot", [B], mybir.dt.int32, isOutput=True)
    gath_out = nc.declare_dram_parameter("gathered", [B], mybir.dt.float32, isOutput=True)

    x_dram = nc.dram_tensor("x_dram", [B], mybir.dt.int32)
    f_shared = nc.dram_tensor("f_shared", [N], mybir.dt.int32, addr_space="Shared")
    f_full = nc.dram_tensor("f_full", [N], mybir.dt.int32)

    father_rows = father.rearrange("(n one) -> n one", one=1)
    ffull_rows = f_full.rearrange("(n one) -> n one", one=1)
    values_rows = values.rearrange("(n one) -> n one", one=1)

    fslice_t = fslice.rearrange("(t p w) -> t p w", p=P, w=W)
    x_t = x_dram.rearrange("(t p w) -> t p w", p=P, w=W)
    froot_t = froot_out.rearrange("(t p w) -> t p w", p=P, w=W)
    gath_t = gath_out.rearrange("(t p w) -> t p w", p=P, w=W)

    core_ids = list(range(N_CORES))

    with (
        nc.sbuf_tensor("offs_sb", [P, W], mybir.dt.int32) as offs_sb,
        nc.sbuf_tensor("g_sb", [P, W], mybir.dt.int32) as g_sb,
        nc.sbuf_tensor("gf_sb", [P, W], mybir.dt.float32) as gf_sb,
        nc.semaphore("s_ld") as s_ld,
        nc.semaphore("s_g") as s_g,
        nc.semaphore("s_st") as s_st,
        nc.semaphore("s_cp") as s_cp,
        nc.semaphore("cc_sem") as cc_sem,
    ):
        GP, SY = mybir.EngineType.Pool, mybir.EngineType.SP

        r_gp = nc.gpsimd.alloc_register("thr_gp")
        r_gp2 = nc.gpsimd.alloc_register("thr_gp2")
        r_sy = nc.sync.alloc_register("thr_sy")
        r_syg = nc.sync.alloc_register("thr_syg")
        r_sy2 = nc.sync.alloc_register("thr_sy2")

        state = {"iters": 0, "cc": 0, "cp": 0}

        def gather_pass(src_tiled, table_rows, out_sb, stores, n_st):
            """Per tile: offs = src tile; out_sb[:,w] = table[offs[:,w]];
            then `stores(it)` issues n_st store DMAs."""
            base = state["iters"]
            with nc.Fori(0, NT, engines=[GP, SY]) as it:
                # thresholds: 16*(base+it+1) and W*16*(base+it+1)
                nc.gpsimd.reg_alu(r_gp, it, base + 1, op=mybir.AluOpType.add)
                nc.gpsimd.reg_alu(r_gp, r_gp, 16, op=mybir.AluOpType.mult)
                nc.gpsimd.reg_alu(r_gp2, r_gp, W, op=mybir.AluOpType.mult)
                nc.sync.reg_alu(r_sy, it, base + 1, op=mybir.AluOpType.add)
                nc.sync.reg_alu(r_sy, r_sy, 16, op=mybir.AluOpType.mult)
                nc.sync.reg_alu(r_syg, r_sy, W, op=mybir.AluOpType.mult)

                nc.sync.dma_start(
                    out=offs_sb[:], in_=src_tiled[bass.ds(it, 1)][0]
                ).then_inc(s_ld, 16)
                nc.gpsimd.wait_ge(s_ld, nc.gpsimd.snap(r_gp))
                for w in range(W):
                    nc.gpsimd.indirect_dma_start(
                        out=out_sb[:, w:w + 1],
                        out_offset=None,
                        in_=table_rows,
                        in_offset=bass.IndirectOffsetOnAxis(
                            ap=offs_sb[:, w:w + 1], axis=0
                        ),
                    ).then_inc(s_g, 16)
                nc.sync.wait_ge(s_g, nc.sync.snap(r_syg))
                stores(it)
                if n_st == 1:
                    nc.sync.wait_ge(s_st, nc.sync.snap(r_sy))
                else:
                    nc.sync.reg_alu(r_sy2, it, 1, op=mybir.AluOpType.add)
                    nc.sync.reg_alu(r_sy2, r_sy2, 16 * n_st, op=mybir.AluOpType.mult)
                    nc.sync.reg_alu(
                        r_sy2, r_sy2, 16 * state["st_prior"], op=mybir.AluOpType.add
                    )
                    nc.sync.wait_ge(s_st, nc.sync.snap(r_sy2))
                nc.gpsimd.wait_ge(s_g, nc.gpsimd.snap(r_gp2))
                nc.multi_engine_barrier([GP, SY])
            state["iters"] += NT

        def st_jump(it):
            nc.sync.dma_start(
                out=x_t[bass.ds(it, 1)][0], in_=g_sb[:]
            ).then_inc(s_st, 16)

        # J1: x = father[fslice]
        gather_pass(fslice_t, father_rows, g_sb, st_jump, 1)

        # J2..JK: AllGather slices -> f_full; x = f_full[x]
        for _ in range(1, n_jumps):
            nc.gpsimd.collective_compute(
                "AllGather",
                mybir.AluOpType.bypass,
                replica_groups=[core_ids],
                ins=[x_dram[:]],
                outs=[f_shared[:]],
            ).then_inc(cc_sem, 1)
            state["cc"] += 1
            nc.sync.wait_ge(cc_sem, state["cc"])
            nc.sync.dma_start(out=f_full[:], in_=f_shared[:]).then_inc(s_cp, 16)
            state["cp"] += 16
            nc.sync.wait_ge(s_cp, state["cp"])
            nc.multi_engine_barrier([GP, SY])
            gather_pass(x_t, ffull_rows, g_sb, st_jump, 1)

        # VG: gathered = values[x]; froot = x
        state["st_prior"] = state["iters"]

        def st_vg(it):
            nc.sync.dma_start(
                out=gath_t[bass.ds(it, 1)][0], in_=gf_sb[:]
            ).then_inc(s_st, 16)
            nc.sync.dma_start(
                out=froot_t[bass.ds(it, 1)][0], in_=offs_sb[:]
            ).then_inc(s_st, 16)

        gather_pass(x_t, values_rows, gf_sb, st_vg, 2)

    return nc


def _run(father_i32, values_f32, n_jumps, trace=False):
    from concourse.bass_utils import run_bass_kernel_spmd

    _enable_dynamic_dma()
    N = len(father_i32)
    B = N // N_CORES
    W = 512
    in_maps = [
        {
            "father": father_i32,
            "values": values_f32,
            "fslice": father_i32[c * B:(c + 1) * B],
        }
        for c in range(N_CORES)
    ]
    last_err = None
    for attempt in range(4):
        try:
            # fresh program per attempt: a wedged exec unit can poison the
            # cached executable/session for the rest of the process
            nc = _build_kernel(N, B, W, n_jumps)
            res = run_bass_kernel_spmd(nc, in_maps, list(range(N_CORES)), trace=trace)
            froot = np.concatenate([res.results[c]["froot"] for c in range(N_CORES)])
            gath = np.concatenate([res.results[c]["gathered"] for c in range(N_CORES)])
            return froot, gath, res
        except Exception as e:  # transient device wedges recover on retry
            last_err = e
            time.sleep(10)
    raise last_err


def _verified(father_i32, values_f32, froot, gath):
    """Exact correctness from input+output alone: roots map to themselves
    (and only roots), froot is a fixed point, froot is path-consistent
    with father, and gathered == values[froot]."""
    idx = np.arange(len(father_i32), dtype=np.int64)
    fr = froot.astype(np.int64)
    fa = father_i32.astype(np.int64)
    if not np.array_equal(fr == idx, fa == idx):
        return False
    if not np.array_equal(fr[fr], fr):
        return False
    if not np.array_equal(fr[fa], fr):
        return False
    if not np.array_equal(gath, values_f32[fr]):
        return False
    return True


def kernel(father: np.ndarray, values: np.ndarray, _trace=False, _jumps=None):
    assert father.shape == (N_EXPECTED,) and values.shape == (N_EXPECTED,), (
        father.shape,
        values.shape,
    )
    out_dtype = father.dtype
    father_i32 = np.ascontiguousarray(father.astype(np.int32))
    values_f32 = np.ascontiguousarray(values.astype(np.float32))

    n_jumps = _jumps or DEFAULT_JUMPS
    for _ in range(3):
        froot, gath, _res = _run(father_i32, values_f32, n_jumps, trace=_trace)
        if _verified(father_i32, values_f32, froot, gath):
            break
        n_jumps += 2  # deeper forest than expected: add doubling passes
    kernel.last_result = _res
    return froot.astype(out_dtype), gath



# revision 12
# speedup vs baseline: 1.1671x; 1.1671x over previous
"""Trainium2 kernel for nn_DisjointSet: pointer-jump densify + value gather.

kernel(father, values) -> (f_conv, gathered), matching the reference:
    f_conv  = fixed point of f <- f[f]   (root id per point)
    gathered = values[f_conv]

Fast path: 8-core SPMD value-telescoping quad gathers. Table T0 holds
16-byte quads (4*father[j], hi16(bits(values[j])), lo16(...), 0). One
indirect-DMA descriptor per point fetches the quad at offset 4*p; each
pass advances the pointer by doubling while carrying a one-step-lagged
value, so 4 passes yield (f^16, values[f^15]) — both outputs from one
gather stream, eliminating classic doubling's separate values pass
(5 passes -> 4). Every transported int is exactly fp32-representable
(4*f has <=24 significant bits; 16-bit halves) because the ncfw
AllGather transport numerically round-trips int32 payloads through
fp32 and rounds anything above 24 mantissa bits (measured). AllGathers
ship 8MB per rank (larger payloads corrupt), quarters reassembled
rank-interleaved into the full quad table.

Outputs are verified host-side from input+output alone; if the fast
path fails verification the kernel falls back to the proven classic
pointer-doubling program, so results are correct by construction.
"""

import time

import numpy as np

import concourse.bass as bass
import concourse.mybir as mybir

P = 128
N_CORES = 8
N_EXPECTED = 16_777_216
DEFAULT_JUMPS = 4  # f^16 pointers, values[f^15] (forest max depth 10)


def _enable_dynamic_dma():
    """walrus in this container defaults to DynamicDMA disabled; the
    vector_dynamic_offsets DGE level is required for indirect DMA
    (per-partition dynamic offsets). Inject the flag into the walrus
    command line."""
    import concourse.bass_utils as bu

    if getattr(bu, "_dyn_dma_patched", False):
        return
    orig = bu.run_command

    def patched(cmd, *a, **kw):
        if cmd and isinstance(cmd[0], str) and "walrus_driver" in cmd[0]:
            if not any(str(c).startswith("--dge-levels") for c in cmd):
                cmd = list(cmd) + [
                    "--dge-levels=io,spill_reload,scalar_dynamic_offset,vector_dynamic_offsets",
                ]
        return orig(cmd, *a, **kw)

    bu.run_command = patched
    bu._dyn_dma_patched = True


def _build_quad(N, B, W, n_jumps):
    """Value-telescoping quad-gather program. N: points; B: per-core
    slice; W: quad gathers per tile; n_jumps: pair passes (>=2)."""
    assert B % (P * W) == 0
    NT = B // (P * W)
    nc = bass.Bass()

    t0 = nc.declare_dram_parameter("t0", [4 * N], mybir.dt.int32, isOutput=False)
    fslice4 = nc.declare_dram_parameter("fslice4", [B], mybir.dt.int32, isOutput=False)
    quads_out = nc.declare_dram_parameter("quads", [4 * B], mybir.dt.int32, isOutput=True)

    xp_dram = nc.dram_tensor("xp_dram", [4 * B], mybir.dt.int32)
    p_sh = nc.dram_tensor("p_sh", [N], mybir.dt.int32, addr_space="Shared")
    p_full = nc.dram_tensor("p_full", [4 * N], mybir.dt.int32)
    # rendezvous scratch: a minimal AllGather doubles as a cross-core
    # barrier (it completes only after every rank has entered it)
    bar_in = nc.dram_tensor("bar_in", [16], mybir.dt.int32)
    bar_out = nc.dram_tensor("bar_out", [16 * N_CORES], mybir.dt.int32, addr_space="Shared")

    t0_rows = t0.rearrange("(n one) -> n one", one=1)
    pfull_rows = p_full.rearrange("(n one) -> n one", one=1)

    fslice_t = fslice4.rearrange("(t p w) -> t p w", p=P, w=W)
    xp_t = xp_dram.rearrange("(t p w) -> t p w", p=P, w=4 * W)
    qout_t = quads_out.rearrange("(t p w) -> t p w", p=P, w=4 * W)

    core_ids = list(range(N_CORES))

    with (
        nc.sbuf_tensor("in_sb", [P, 4 * W], mybir.dt.int32) as in_sb,
        nc.sbuf_tensor("gp_sb", [P, 4 * W], mybir.dt.int32) as gp_sb,
        nc.semaphore("s_ld") as s_ld,
        nc.semaphore("s_g") as s_g,
        nc.semaphore("s_st") as s_st,
        nc.semaphore("s_cp") as s_cp,
        nc.semaphore("cc_sem") as cc_sem,
    ):
        GP, SY = mybir.EngineType.Pool, mybir.EngineType.SP

        r_gp = nc.gpsimd.alloc_register("thr_gp")
        r_gp2 = nc.gpsimd.alloc_register("thr_gp2")
        r_sy = nc.sync.alloc_register("thr_sy")
        r_syg = nc.sync.alloc_register("thr_syg")

        state = {"iters": 0, "cc": 0, "cp": 0}

        def gather_pass(src_tiled, table_rows, out_tiled, first):
            base = state["iters"]
            with nc.Fori(0, NT, engines=[GP, SY]) as it:
                nc.gpsimd.reg_alu(r_gp, it, base + 1, op=mybir.AluOpType.add)
                nc.gpsimd.reg_alu(r_gp, r_gp, 16, op=mybir.AluOpType.mult)
                nc.gpsimd.reg_alu(r_gp2, r_gp, W, op=mybir.AluOpType.mult)
                nc.sync.reg_alu(r_sy, it, base + 1, op=mybir.AluOpType.add)
                nc.sync.reg_alu(r_sy, r_sy, 16, op=mybir.AluOpType.mult)
                nc.sync.reg_alu(r_syg, r_sy, W, op=mybir.AluOpType.mult)

                if first:
                    nc.sync.dma_start(
                        out=in_sb[:, 0:W], in_=src_tiled[bass.ds(it, 1)][0]
                    ).then_inc(s_ld, 16)
                else:
                    nc.sync.dma_start(
                        out=in_sb[:], in_=src_tiled[bass.ds(it, 1)][0]
                    ).then_inc(s_ld, 16)
                nc.gpsimd.wait_ge(s_ld, nc.gpsimd.snap(r_gp))
                for w in range(W):
                    oc = w if first else 4 * w
                    nc.gpsimd.indirect_dma_start(
                        out=gp_sb[:, 4 * w:4 * w + 4],
                        out_offset=None,
                        in_=table_rows,
                        in_offset=bass.IndirectOffsetOnAxis(
                            ap=in_sb[:, oc:oc + 1], axis=0
                        ),
                    ).then_inc(s_g, 16)
                nc.sync.wait_ge(s_g, nc.sync.snap(r_syg))
                nc.sync.dma_start(
                    out=out_tiled[bass.ds(it, 1)][0], in_=gp_sb[:]
                ).then_inc(s_st, 16)
                nc.sync.wait_ge(s_st, nc.sync.snap(r_sy))
                nc.gpsimd.wait_ge(s_g, nc.gpsimd.snap(r_gp2))
                nc.multi_engine_barrier([GP, SY])
            state["iters"] += NT

        # init rendezvous scratch so the dummy AllGather reads defined data
        nc.sync.dma_start(out=bar_in[:], in_=t0[0:16]).then_inc(s_cp, 16)
        state["cp"] += 16
        nc.sync.wait_ge(s_cp, state["cp"])
        nc.multi_engine_barrier([GP, SY])

        # P1: quads = T0[4*father[slice]]
        last_out = qout_t if n_jumps == 1 else xp_t
        gather_pass(fslice_t, t0_rows, last_out, first=True)

        # P2..PK: AllGather quad slices (four 8MB quarters through the
        # 64MB shared buffer) -> p_full; quads = p_full[ptr]
        Q = B  # int32 elements per quarter (= 8MB)
        xp_q = xp_dram.rearrange("(q k) -> q k", k=Q)
        sh_r = p_sh.rearrange("(r k) -> r k", k=Q)
        pf_rq = p_full.rearrange("(rq k) -> rq k", k=Q)
        for k in range(1, n_jumps):
            for q in range(4):
                nc.gpsimd.collective_compute(
                    "AllGather",
                    mybir.AluOpType.bypass,
                    replica_groups=[core_ids],
                    ins=[xp_q[bass.ds(q, 1)][0]],
                    outs=[p_sh[:]],
                ).then_inc(cc_sem, 1)
                state["cc"] += 1
                nc.sync.wait_ge(cc_sem, state["cc"])
                for r in range(N_CORES):
                    nc.sync.dma_start(
                        out=pf_rq[bass.ds(4 * r + q, 1)][0],
                        in_=sh_r[bass.ds(r, 1)][0],
                    ).then_inc(s_cp, 16)
                    state["cp"] += 16
                nc.sync.wait_ge(s_cp, state["cp"])
                # p_sh is reused next quarter: every rank must finish its
                # copy-out before any rank's next AllGather overwrites
                # peers' p_sh. Local barrier syncs gpsimd with the local
                # copies; the dummy AllGather is the cross-core rendezvous.
                nc.multi_engine_barrier([GP, SY])
                nc.gpsimd.collective_compute(
                    "AllGather",
                    mybir.AluOpType.bypass,
                    replica_groups=[core_ids],
                    ins=[bar_in[:]],
                    outs=[bar_out[:]],
                ).then_inc(cc_sem, 1)
                state["cc"] += 1
                nc.gpsimd.wait_ge(cc_sem, state["cc"])
                nc.sync.wait_ge(cc_sem, state["cc"])
            nc.multi_engine_barrier([GP, SY])
            last_out = qout_t if k == n_jumps - 1 else xp_t
            gather_pass(xp_t, pfull_rows, last_out, first=False)

    return nc


def _build_legacy(N, B, W, n_jumps):
    """Classic pointer doubling + separate values gather (proven exact)."""
    assert B % (P * W) == 0
    NT = B // (P * W)
    nc = bass.Bass()

    father = nc.declare_dram_parameter("father", [N], mybir.dt.int32, isOutput=False)
    values = nc.declare_dram_parameter("values", [N], mybir.dt.float32, isOutput=False)
    fslice = nc.declare_dram_parameter("fslice", [B], mybir.dt.int32, isOutput=False)
    froot_out = nc.declare_dram_parameter("froot", [B], mybir.dt.int32, isOutput=True)
    gath_out = nc.declare_dram_parameter("gathered", [B], mybir.dt.float32, isOutput=True)

    x_dram = nc.dram_tensor("x_dram", [B], mybir.dt.int32)
    f_shared = nc.dram_tensor("f_shared", [N], mybir.dt.int32, addr_space="Shared")
    f_full = nc.dram_tensor("f_full", [N], mybir.dt.int32)

    father_rows = father.rearrange("(n one) -> n one", one=1)
    ffull_rows = f_full.rearrange("(n one) -> n one", one=1)
    values_rows = values.rearrange("(n one) -> n one", one=1)

    fslice_t = fslice.rearrange("(t p w) -> t p w", p=P, w=W)
    x_t = x_dram.rearrange("(t p w) -> t p w", p=P, w=W)
    froot_t = froot_out.rearrange("(t p w) -> t p w", p=P, w=W)
    gath_t = gath_out.rearrange("(t p w) -> t p w", p=P, w=W)

    core_ids = list(range(N_CORES))

    with (
        nc.sbuf_tensor("offs_sb", [P, W], mybir.dt.int32) as offs_sb,
        nc.sbuf_tensor("g_sb", [P, W], mybir.dt.int32) as g_sb,
        nc.sbuf_tensor("gf_sb", [P, W], mybir.dt.float32) as gf_sb,
        nc.semaphore("s_ld") as s_ld,
        nc.semaphore("s_g") as s_g,
        nc.semaphore("s_st") as s_st,
        nc.semaphore("s_cp") as s_cp,
        nc.semaphore("cc_sem") as cc_sem,
    ):
        GP, SY = mybir.EngineType.Pool, mybir.EngineType.SP

        r_gp = nc.gpsimd.alloc_register("thr_gp")
        r_gp2 = nc.gpsimd.alloc_register("thr_gp2")
        r_sy = nc.sync.alloc_register("thr_sy")
        r_syg = nc.sync.alloc_register("thr_syg")
        r_sy2 = nc.sync.alloc_register("thr_sy2")

        state = {"iters": 0, "cc": 0, "cp": 0}

        def gather_pass(src_tiled, table_rows, out_sb, stores, n_st):
            base = state["iters"]
            with nc.Fori(0, NT, engines=[GP, SY]) as it:
                nc.gpsimd.reg_alu(r_gp, it, base + 1, op=mybir.AluOpType.add)
                nc.gpsimd.reg_alu(r_gp, r_gp, 16, op=mybir.AluOpType.mult)
                nc.gpsimd.reg_alu(r_gp2, r_gp, W, op=mybir.AluOpType.mult)
                nc.sync.reg_alu(r_sy, it, base + 1, op=mybir.AluOpType.add)
                nc.sync.reg_alu(r_sy, r_sy, 16, op=mybir.AluOpType.mult)
                nc.sync.reg_alu(r_syg, r_sy, W, op=mybir.AluOpType.mult)

                nc.sync.dma_start(
                    out=offs_sb[:], in_=src_tiled[bass.ds(it, 1)][0]
                ).then_inc(s_ld, 16)
                nc.gpsimd.wait_ge(s_ld, nc.gpsimd.snap(r_gp))
                for w in range(W):
                    nc.gpsimd.indirect_dma_start(
                        out=out_sb[:, w:w + 1],
                        out_offset=None,
                        in_=table_rows,
                        in_offset=bass.IndirectOffsetOnAxis(
                            ap=offs_sb[:, w:w + 1], axis=0
                        ),
                    ).then_inc(s_g, 16)
                nc.sync.wait_ge(s_g, nc.sync.snap(r_syg))
                stores(it)
                if n_st == 1:
                    nc.sync.wait_ge(s_st, nc.sync.snap(r_sy))
                else:
                    nc.sync.reg_alu(r_sy2, it, 1, op=mybir.AluOpType.add)
                    nc.sync.reg_alu(r_sy2, r_sy2, 16 * n_st, op=mybir.AluOpType.mult)
                    nc.sync.reg_alu(
                        r_sy2, r_sy2, 16 * state["st_prior"], op=mybir.AluOpType.add
                    )
                    nc.sync.wait_ge(s_st, nc.sync.snap(r_sy2))
                nc.gpsimd.wait_ge(s_g, nc.gpsimd.snap(r_gp2))
                nc.multi_engine_barrier([GP, SY])
            state["iters"] += NT

        def st_jump(it):
            nc.sync.dma_start(
                out=x_t[bass.ds(it, 1)][0], in_=g_sb[:]
            ).then_inc(s_st, 16)

        gather_pass(fslice_t, father_rows, g_sb, st_jump, 1)

        for _ in range(1, n_jumps):
            nc.gpsimd.collective_compute(
                "AllGather",
                mybir.AluOpType.bypass,
                replica_groups=[core_ids],
                ins=[x_dram[:]],
                outs=[f_shared[:]],
            ).then_inc(cc_sem, 1)
            state["cc"] += 1
            nc.sync.wait_ge(cc_sem, state["cc"])
            nc.sync.dma_start(out=f_full[:], in_=f_shared[:]).then_inc(s_cp, 16)
            state["cp"] += 16
            nc.sync.wait_ge(s_cp, state["cp"])
            nc.multi_engine_barrier([GP, SY])
            gather_pass(x_t, ffull_rows, g_sb, st_jump, 1)

        state["st_prior"] = state["iters"]

        def st_vg(it):
            nc.sync.dma_start(
                out=gath_t[bass.ds(it, 1)][0], in_=gf_sb[:]
            ).then_inc(s_st, 16)
            nc.sync.dma_start(
                out=froot_t[bass.ds(it, 1)][0], in_=offs_sb[:]
            ).then_inc(s_st, 16)

        gather_pass(x_t, values_rows, gf_sb, st_vg, 2)

    return nc


def _run_quad(t0_np, fslice4_list, n_jumps, trace=False):
    from concourse.bass_utils import run_bass_kernel_spmd

    _enable_dynamic_dma()
    N = N_EXPECTED
    B = N // N_CORES
    W = 512
    in_maps = [
        {"t0": t0_np, "fslice4": fslice4_list[c]} for c in range(N_CORES)
    ]
    nc = _build_quad(N, B, W, n_jumps)
    res = run_bass_kernel_spmd(nc, in_maps, list(range(N_CORES)), trace=trace)
    quads = np.concatenate([res.results[c]["quads"] for c in range(N_CORES)])
    froot = quads[0::4] >> 2
    hi = np.ascontiguousarray(quads[1::4]).astype(np.uint32)
    lo = np.ascontiguousarray(quads[2::4]).astype(np.uint32)
    gath = ((hi << np.uint32(16)) | lo).view(np.float32)
    return froot, gath, res


def _run_legacy(father_i32, values_f32, n_jumps, trace=False):
    from concourse.bass_utils import run_bass_kernel_spmd

    _enable_dynamic_dma()
    N = len(father_i32)
    B = N // N_CORES
    W = 512
    in_maps = [
        {
            "father": father_i32,
            "values": values_f32,
            "fslice": father_i32[c * B:(c + 1) * B],
        }
        for c in range(N_CORES)
    ]
    last_err = None
    for attempt in range(4):
        try:
            nc = _build_legacy(N, B, W, n_jumps)
            res = run_bass_kernel_spmd(nc, in_maps, list(range(N_CORES)), trace=trace)
            froot = np.concatenate([res.results[c]["froot"] for c in range(N_CORES)])
            gath = np.concatenate([res.results[c]["gathered"] for c in range(N_CORES)])
            return froot, gath, res
        except Exception as e:  # transient device wedges recover on retry
            last_err = e
            time.sleep(10)
    raise last_err


def _verified(father_i32, values_f32, froot, gath):
    """Exact correctness from input+output alone: roots map to themselves
    (and only roots), froot is a fixed point, froot is path-consistent
    with father, and gathered == values[froot]."""
    idx = np.arange(len(father_i32), dtype=np.int64)
    fr = froot.astype(np.int64)
    fa = father_i32.astype(np.int64)
    if not np.array_equal(fr == idx, fa == idx):
        return False
    if not np.array_equal(fr[fr], fr):
        return False
    if not np.array_equal(fr[fa], fr):
        return False
    if not np.array_equal(gath, values_f32[fr]):
        return False
    return True


def kernel(father: np.ndarray, values: np.ndarray, _trace=False, _jumps=None):
    assert father.shape == (N_EXPECTED,) and values.shape == (N_EXPECTED,), (
        father.shape,
        values.shape,
    )
    out_dtype = father.dtype
    father_i32 = np.ascontiguousarray(father.astype(np.int32))
    values_f32 = np.ascontiguousarray(values.astype(np.float32))

    n_jumps = _jumps or DEFAULT_JUMPS

    # fast path: quad telescoping (every transported int fp32-exact)
    try:
        vu = values_f32.view(np.uint32)
        t0 = np.empty(4 * N_EXPECTED, dtype=np.int32)
        t0[0::4] = father_i32 * 4
        t0[1::4] = (vu >> np.uint32(16)).astype(np.int32)
        t0[2::4] = (vu & np.uint32(0xFFFF)).astype(np.int32)
        t0[3::4] = 0
        B = N_EXPECTED // N_CORES
        fslice4 = [t0[0::4][c * B:(c + 1) * B].copy() for c in range(N_CORES)]
        froot, gath, _res = _run_quad(t0, fslice4, n_jumps, trace=_trace)
        if _verified(father_i32, values_f32, froot, gath):
            kernel.last_result = _res
            return froot.astype(out_dtype), gath
    except Exception:
        pass

    # fallback: proven classic doubling (with its own verify-retry loop)
    for _ in range(3):
        froot, gath, _res = _run_legacy(father_i32, values_f32, n_jumps, trace=_trace)
        if _verified(father_i32, values_f32, froot, gath):
            break
        n_jumps += 2
    kernel.last_result = _res
    return froot.astype(out_dtype), gath
